# revision 1
# baseline (speedup 1.0000x reference)
"""Trainium2 Bass kernel for nn_DynamicDWConv.

Math note: the reference applies nn.Softmax over dim=1 of a (b*c, 1, K, K)
tensor -- a singleton axis -- so the "dynamic" depthwise weights are exactly
1.0 everywhere and w1/b1/w2/b2 have no effect on the output. The computation
reduces to:

    y[b, c, h, w] = x[b, c, h, w] + bias[c] + sum_{|dh|<=1, |dw|<=1} x[b, c, h+dh, w+dw]

(zero padding at the borders). This is a memory-bound 3x3 box-sum stencil.

Per core (4 samples of the batch, data-parallel over batch across 8 cores),
variant B layout ("h-pairs", the default):
  - Supertile = 8 consecutive channels (2 quads) x all 4 samples. SBUF
    partitions = (c4 in 0..3, h2 in 0..31) with h = 2*h2 + hl; free dim =
    (quad, b, hl, w_padded). Each DMA descriptor covers two H rows
    contiguously (528B in / 512B out), keeping the SDMA engines at full rate
    (<512B forces read-modify-write).
  - The H-direction 3-tap sum becomes 4 exact-integer stationary matrices,
    block-diagonal over the 32 h2 rows per c4 block:
        out(hl=0)[h2] = X0[h2] + X1[h2] + X1[h2-1]   -> I @ X0 + (I+SUP) @ X1
        out(hl=1)[h2] = X0[h2] + X1[h2] + X0[h2+1]   -> (I+SUB) @ X0 + I @ X1
    The W-direction sum is 3 W-shifted accumulating matmuls per term (x is
    zero-padded by one column in W on the host so every matmul keeps
    free-dim = 256, the full-rate threshold for fp32r). The "+x" term rides
    the unshifted matmul by doubling the identity (2I). 12 matmuls and 2 PSUM
    banks per quad; matmuls are grouped by stationary across the two quads of
    a supertile to minimize PE weight reloads.
  - PSUM -> SBUF move with fused per-partition bias: ScalarE activation
    (Identity, bias AP) for the hl=0 half, VectorE tensor_scalar add for hl=1.
  - DMA: input loads on the SP HWDGE ring (nc.sync), output stores on the ACT
    HWDGE ring (nc.scalar), splitting descriptor generation across both rings.

Variant A (channel-pairs, kept for comparison): partitions = (c2, h), one
tridiagonal stationary, 3 matmuls per 2-channel supertile, but 256B DMA runs.
"""

import os

import numpy as np

B_TOTAL = 32
B_CORE = 4
N_CORES = 8
C = 256
H = 64
W = 64
WP = W + 2  # zero-padded width (1 zero each side)
NPAIR = C // 2  # variant A: 128 channel-pairs per sample
NGRP = C // 4  # variant B: 64 channel-quads per sample
NSUP = C // 8  # variant B: 32 supertiles (2 quads each) per sample

_nc_cache = {}
last_results = None  # BassKernelResults of the most recent run (for test harness)


def _variant():
    return os.environ.get("KERNEL_VARIANT", "B").upper()


def _build_nc_a(hwloop=1):
    import concourse.bacc as bacc
    import concourse.mybir as mybir
    from concourse import tile

    # Bacc (not plain Bass): its compile() runs move_matmul_waits_to_ldweights
    # + generate_event_semaphores, which split semaphore waits to satisfy the
    # TRN2 "at most 1 wait per instruction" encoding constraint.
    nc = bacc.Bacc()

    f32 = mybir.dt.float32
    f32r = mybir.dt.float32r

    xp = nc.dram_tensor("xp", [B_CORE, C, H, WP], f32r, kind="ExternalInput")
    # consts packed into one tensor -> one DMA -> one semaphore:
    # cols 0:128 = T, 128:256 = T+I, 256:256+NPAIR = bias columns
    consts_d = nc.dram_tensor("consts", [128, 256 + NPAIR], f32r, kind="ExternalInput")
    y = nc.dram_tensor("y", [B_CORE, C, H, W], f32, kind="ExternalOutput")

    x_re = xp[:].rearrange("b (pair c2) h w -> pair (c2 h) b w", c2=2)
    y_re = y[:].rearrange("b (pair c2) h w -> pair (c2 h) b w", c2=2)

    ident = mybir.ActivationFunctionType.Identity

    with tile.TileContext(nc) as tc:
        with (
            tc.tile_pool(name="consts", bufs=1) as consts,
            tc.tile_pool(name="xin", bufs=12) as x_pool,
            tc.tile_pool(name="yout", bufs=12) as y_pool,
            tc.tile_pool(name="acc", bufs=8, space="PSUM") as psum_pool,
        ):
            c_sb = consts.tile([128, 256 + NPAIR], f32r)
            nc.sync.dma_start(c_sb[:], consts_d[:])
            t_sb = c_sb[:, 0:128]
            ti_sb = c_sb[:, 128:256]
            bias_sb = c_sb[:, 256 : 256 + NPAIR].bitcast(f32)

            # Warmup ops touching the consts on PE/ACT/DVE: later instructions
            # on those engines depend on the const DMA transitively through
            # program order instead of extra semaphore waits.
            warm = psum_pool.tile([128, B_CORE, W], f32, tag="ps")
            nc.tensor.matmul(
                warm[:].rearrange("p b w -> p (b w)")[:, 0:128], ti_sb, t_sb,
                start=True, stop=True,
            )
            scratch = consts.tile([128, 2], f32)
            nc.scalar.activation(
                scratch[:, 0:1], bias_sb[:, 0:1],
                mybir.ActivationFunctionType.Copy,
            )
            nc.vector.tensor_copy(scratch[:, 1:2], bias_sb[:, 0:1])

            for _rep in range(hwloop):
                for pair in range(NPAIR):
                    x_sb = x_pool.tile([128, B_CORE, WP], f32r)
                    nc.sync.dma_start(x_sb[:], x_re[pair])

                    # x data at cols 1..64 (w = col-1), zeros at cols 0 and 65;
                    # psum col k = y[w=k]: TI@xs[k+1] + T@xs[k] + T@xs[k+2]
                    ps = psum_pool.tile([128, B_CORE, W], f32, tag="ps")
                    pf = ps[:].rearrange("p b w -> p (b w)")
                    nc.tensor.matmul(pf, ti_sb, x_sb[:, :, 1 : 1 + W], start=True, stop=False)
                    nc.tensor.matmul(pf, t_sb, x_sb[:, :, 0:W], start=False, stop=False)
                    nc.tensor.matmul(pf, t_sb, x_sb[:, :, 2 : 2 + W], start=False, stop=True)

                    y_sb = y_pool.tile([128, B_CORE, W], f32)
                    bias_ap = bias_sb[:, pair : pair + 1]
                    if pair % 2 == 0:
                        nc.scalar.activation(y_sb[:], ps[:], ident, bias=bias_ap, scale=1.0)
                    else:
                        nc.vector.tensor_scalar_add(y_sb[:], ps[:], bias_ap)

                    nc.scalar.dma_start(y_re[pair], y_sb[:])

    nc.compile()
    return nc


def _build_nc_b(hwloop=1):
    import concourse.bacc as bacc
    import concourse.mybir as mybir
    from concourse import tile

    nc = bacc.Bacc()

    f32 = mybir.dt.float32
    f32r = mybir.dt.float32r
    bf16 = mybir.dt.bfloat16

    xp = nc.dram_tensor("xp", [B_CORE, C, H, WP], f32r, kind="ExternalInput")
    # consts packed into one tensor -> one DMA -> one semaphore. Stationary
    # matrices (exact 0/1/2 entries) are f32r to match the moving operand
    # (walrus birverifier rejects mixed-dtype matmuls): cols 0:128 I,
    # 128:256 2I, 256:384 I+SUP, 384:512 I+SUB, 512:512+NGRP bias columns.
    consts_d = nc.dram_tensor("consts", [128, 512 + NGRP], f32r, kind="ExternalInput")
    y = nc.dram_tensor("y", [B_CORE, C, H, W], f32, kind="ExternalOutput")

    # supertile = 2 quads (8 channels) x 4 samples x both hl planes
    x_re = xp[:].rearrange(
        "b (sup q c4) (h2 hl) w -> sup (c4 h2) q b hl w", q=2, c4=4, hl=2
    )
    y_re = y[:].rearrange(
        "b (sup q c4) (h2 hl) w -> sup (c4 h2) q b hl w", q=2, c4=4, hl=2
    )

    ident = mybir.ActivationFunctionType.Identity

    with tile.TileContext(nc) as tc:
        with (
            tc.tile_pool(name="consts", bufs=1) as consts,
            # bufs=6 is the swept optimum (4 or 8+ measurably worse)
            tc.tile_pool(name="xin", bufs=6) as x_pool,
            tc.tile_pool(name="yout", bufs=6) as y_pool,
            tc.tile_pool(name="acc", bufs=4, space="PSUM") as psum_pool,
        ):
            c_sb = consts.tile([128, 512 + NGRP], f32r)
            nc.sync.dma_start(c_sb[:], consts_d[:])
            m_i = c_sb[:, 0:128]
            m_2i = c_sb[:, 128:256]
            m_isup = c_sb[:, 256:384]
            m_isub = c_sb[:, 384:512]
            bias_sb = c_sb[:, 512 : 512 + NGRP].bitcast(f32)

            # Warmups: make later instructions depend on the const DMA through
            # engine program order (TRN2 allows ~1 semaphore wait per instr).
            warm = psum_pool.tile([128, B_CORE, W], f32, tag="ps0")
            nc.tensor.matmul(
                warm[:].rearrange("p b w -> p (b w)")[:, 0:128], m_i, m_2i,
                start=True, stop=True,
            )
            scratch = consts.tile([128, 2], f32)
            nc.scalar.activation(
                scratch[:, 0:1], bias_sb[:, 0:1],
                mybir.ActivationFunctionType.Copy,
            )
            nc.vector.tensor_copy(scratch[:, 1:2], bias_sb[:, 0:1])

            for _rep in range(hwloop):
                for sup in range(NSUP):
                    x_sb = x_pool.tile([128, 2, B_CORE, 2, WP], f32r)
                    # per-quad DMAs: the DMA lowering balances at most 3 AP
                    # dims, and (quad, b) don't merge in DRAM. Each quad pair
                    # is split across the two HWDGE rings so per-supertile
                    # descriptor generation runs on both in parallel.
                    e_in = (nc.sync, nc.scalar) if sup % 2 == 0 else (nc.scalar, nc.sync)
                    e_in[0].dma_start(x_sb[:, 0], x_re[sup][:, 0])
                    e_in[1].dma_start(x_sb[:, 1], x_re[sup][:, 1])

                    # x data at w-cols 1..64, zeros at cols 0 and 65;
                    # psum col k = y[w=k] (taps read cols k, k+1, k+2).
                    # NOTE: fp32r matmuls are kept at FD=256 — an FD=512
                    # fp32r moving operand crashes the exec unit on HW
                    # (fp32r appears to stream each column twice, so the
                    # effective per-instruction stream limit is 256, not the
                    # fp32 512).
                    def xs(q, hl, s):
                        return x_sb[:, q, :, hl, s : s + W]

                    pss = [
                        [
                            psum_pool.tile(
                                [128, B_CORE, W], f32,
                                tag=f"ps{hl}", name=f"ps_{sup}_{q}_{hl}",
                            )
                            for hl in range(2)
                        ]
                        for q in range(2)
                    ]
                    pf = [
                        [pss[q][hl][:].rearrange("p b w -> p (b w)") for hl in range(2)]
                        for q in range(2)
                    ]
                    started = [[False, False], [False, False]]

                    def mm(q, hl, mat, rhs, stop=False):
                        nc.tensor.matmul(
                            pf[q][hl], mat, rhs,
                            start=not started[q][hl], stop=stop,
                        )
                        started[q][hl] = True

                    # grouped by stationary (across both quads) to minimize
                    # PE weight reloads: ISUP, ISUB, I, 2I
                    for q in range(2):
                        for s in range(3):
                            mm(q, 0, m_isup, xs(q, 1, s))
                    for q in range(2):
                        for s in range(3):
                            mm(q, 1, m_isub, xs(q, 0, s))
                    for q in range(2):
                        mm(q, 0, m_i, xs(q, 0, 0))
                        mm(q, 0, m_i, xs(q, 0, 2))
                        mm(q, 1, m_i, xs(q, 1, 0))
                        mm(q, 1, m_i, xs(q, 1, 2))
                    for q in range(2):
                        mm(q, 0, m_2i, xs(q, 0, 1), stop=True)
                        mm(q, 1, m_2i, xs(q, 1, 1), stop=True)

                    y_sb = y_pool.tile([128, 2, B_CORE, 2, W], f32)
                    for q in range(2):
                        bias_ap = bias_sb[:, 2 * sup + q : 2 * sup + q + 1]
                        nc.scalar.activation(
                            y_sb[:, q, :, 0, :], pss[q][0][:], ident,
                            bias=bias_ap, scale=1.0,
                        )
                        nc.vector.tensor_scalar_add(
                            y_sb[:, q, :, 1, :], pss[q][1][:], bias_ap
                        )

                    # stores all on the SP ring: they carry compute-result
                    # dependencies anyway, and this keeps the ACT ring (which
                    # also runs the activations) at half the issue load
                    nc.sync.dma_start(y_re[sup][:, 0], y_sb[:, 0])
                    nc.sync.dma_start(y_re[sup][:, 1], y_sb[:, 1])

    nc.compile()
    return nc


def _get_nc(hwloop=1, variant=None):
    variant = variant or _variant()
    key = ("nc", variant, hwloop)
    if key not in _nc_cache:
        _nc_cache[key] = (
            _build_nc_b(hwloop) if variant == "B" else _build_nc_a(hwloop)
        )
    return _nc_cache[key]


def _host_prep(x, bias, variant=None):
    """Build per-core input maps from the full inputs."""
    variant = variant or _variant()
    x = np.ascontiguousarray(x, dtype=np.float32)
    bias = np.ascontiguousarray(bias, dtype=np.float32)

    if variant == "A":
        # T: block-diagonal tridiagonal (3-tap H sum); TI = T + I ("+x" fold)
        t64 = np.zeros((64, 64), dtype=np.float32)
        for d in (-1, 0, 1):
            t64 += np.eye(64, k=d, dtype=np.float32)
        tmat = np.zeros((128, 128), dtype=np.float32)
        tmat[:64, :64] = t64
        tmat[64:, 64:] = t64
        timat = tmat + np.eye(128, dtype=np.float32)
        biasc = np.empty((128, NPAIR), dtype=np.float32)
        biasc[:64, :] = bias[0::2][None, :]
        biasc[64:, :] = bias[1::2][None, :]
        consts = np.concatenate([tmat, timat, biasc], axis=1)
    else:
        # Block-diagonal (4 blocks of 32 h2-rows) matrices, lhsT orientation:
        # out[m] = sum_k lhsT[k, m] rhs[k].
        eye32 = np.eye(32, dtype=np.float32)
        sup32 = np.eye(32, k=1, dtype=np.float32)  # [k, k+1]: out[h2] += in[h2-1]
        sub32 = np.eye(32, k=-1, dtype=np.float32)  # [k, k-1]: out[h2] += in[h2+1]

        def bd(block):
            m = np.zeros((128, 128), dtype=np.float32)
            for i in range(4):
                m[i * 32 : (i + 1) * 32, i * 32 : (i + 1) * 32] = block
            return m

        biasc = np.empty((128, NGRP), dtype=np.float32)
        for c4 in range(4):
            biasc[c4 * 32 : (c4 + 1) * 32, :] = bias[c4::4][None, :]
        consts = np.concatenate(
            [bd(eye32), bd(2.0 * eye32), bd(eye32 + sup32), bd(eye32 + sub32), biasc],
            axis=1,
        )

    xs = x.reshape(N_CORES, B_CORE, C, H, W)
    xp = np.zeros((N_CORES, B_CORE, C, H, WP), dtype=np.float32)
    xp[..., 1 : W + 1] = xs

    in_maps = [
        {"xp": np.ascontiguousarray(xp[k]), "consts": consts}
        for k in range(N_CORES)
    ]
    return in_maps


def kernel(x, w1=None, b1=None, w2=None, b2=None, bias=None, **_unused):
    global last_results
    from concourse.bass_utils import run_bass_kernel_spmd

    if bias is None:
        bias = np.zeros((C,), dtype=np.float32)

    nc = _get_nc()
    in_maps = _host_prep(x, bias)
    trace = bool(int(os.environ.get("KERNEL_TRACE", "0")))
    try:
        res = run_bass_kernel_spmd(
            nc, in_maps, core_ids=list(range(N_CORES)), trace=trace
        )
    except ModuleNotFoundError:
        # Tracing under axon needs antenv.axon_hooks, which some client
        # environments lack; rerun with tracing disabled rather than dying.
        os.environ["BASS_NEVER_TRACE"] = "1"
        try:
            res = run_bass_kernel_spmd(
                nc, in_maps, core_ids=list(range(N_CORES)), trace=False
            )
        finally:
            os.environ.pop("BASS_NEVER_TRACE", None)
    last_results = res
    y = np.concatenate(
        [res.results[k]["y"].reshape(B_CORE, C, H, W) for k in range(N_CORES)],
        axis=0,
    )
    return y

